# revision 27
# baseline (speedup 1.0000x reference)
"""Trainium2 Bass kernel for attention pooling over graph segments.

Reference computation (per node i with segment b = batch[i]):
    h   = LN(leaky_relu(feat @ W1 + b1)) * g1 + beta1
    att = exp(h @ W2 + b2)
    s_b = segment_sum(att);  att_n = att / s_b
    out_b = segment_sum(att_n[:, :, None] * feat[:, None, :])   # [B, H, D]
    o = LN(lrelu(out @ W3 + b3)) ; o = LN(lrelu(o @ W4 + b4))

Strategy (8 cores, data parallel by graph):
  - 512 graphs per core, grouped into 32 windows of 16 graphs.
  - Nodes are re-packed on host so each window's nodes start 128-aligned
    (padded with zero rows). A one-hot "window-local graph id" C[n, 0:16]
    plus a ones column are packed after the 128 feat columns.
  - att normalization folds into the output: out_b = (sum att*feat)/s_b.
  - Per 128-node chunk, lhsT = C (x) att (weighted one-hot [128, 16*8]) and
    one matmul against [feat | ones] accumulates both sum(att*feat) and s.

This walrus encodes at most one semaphore wait per instruction;
_split_multiwaits() hoists extra waits onto InstEventSemaphore carriers.

Perf notes: fp32 matmuls stream at 2 cycles/column, bf16 at 1 — the
node-MLP and segment matmuls run in bf16 (f32 PSUM accumulation).  All
ScalarE functions used (parametric relu, copy, ln, exp, square) live in
one activation-table set, and rstd = exp(-0.5*ln(var+eps)) keeps sqrt
off the engine, so the ~1.3us ACT_TABLE_LOAD never recurs.
"""

import os
import sys

import numpy as np

try:  # make concourse importable in bare environments
    import concourse  # noqa: F401
except ImportError:  # pragma: no cover
    sys.path.insert(0, "/opt/trn_rl_repo")

NUM_GRAPHS = 4096
NC_CORES = 8
WG = 16  # graphs per window
D = 128
H = 8
CH = 64  # hidden channels
EPS = 1e-6
SLOPE = 0.01

# packed f32 constant column offsets (wpk, [128, PKW])
PK_CWN = 0         # [128, 8]  -colsum(W2g), broadcast
PK_B1 = 8          # [64, 1]
PK_ID = 9          # [128, 128] identity
PK_B2 = 137        # [128, 8]  b2 + beta1@W2, broadcast
PKW = 145

LAST_RESULT = None  # BassKernelResults of the most recent run (for test.py)


def _prep_shards(feat, seg, bf16):
    """Window-pad nodes per core; build fpx = [feat | ones | C] and feat^T."""
    bounds = np.searchsorted(seg, np.arange(NUM_GRAPHS + 1))
    wstart = bounds[::WG]  # 257 entries
    wcnt = np.diff(wstart)
    nwin_per_core = NUM_GRAPHS // WG // NC_CORES  # 32
    NW = max(128, int(-(-int(wcnt.max()) // 128)) * 128)
    NTOT = nwin_per_core * NW
    fpx_all = []
    ft_all = []
    for k in range(NC_CORES):
        fpx = np.zeros((NTOT, D + 1 + WG), np.float32)
        fpx[:, D] = 1.0  # ones column (harmless on pad rows; C gates them)
        for j in range(nwin_per_core):
            w = k * nwin_per_core + j
            s, e = int(wstart[w]), int(wstart[w + 1])
            n = e - s
            if n == 0:
                continue
            fpx[j * NW : j * NW + n, :D] = feat[s:e]
            gl = (seg[s:e] - w * WG).astype(np.int64)
            fpx[j * NW + np.arange(n), D + 1 + gl] = 1.0
        ft = np.ascontiguousarray(fpx[:, :D].T).astype(bf16)
        fpx_all.append(fpx.astype(bf16))
        ft_all.append(ft)
    return fpx_all, ft_all, NW, NTOT


def _build_program(NW, NTOT, host):
    import concourse.bass as bass
    import concourse.tile as tile
    from concourse import mybir

    f32 = mybir.dt.float32
    bf16 = mybir.dt.bfloat16
    AF = mybir.ActivationFunctionType
    OP = mybir.AluOpType

    GPC = NUM_GRAPHS // NC_CORES  # 512 graphs per core
    CPW = NW // 128  # chunks per window
    NCHUNK = NTOT // 128
    G = 8  # chunks per batch (1024 nodes)
    NB = NCHUNK // G
    FPW = D + 1 + WG  # fpx row width: feat | ones | C
    PK2W = H * D + D + 6 * D  # w3 | w4 | 6 broadcast vectors

    use_b2 = bool(np.any(host["b2p"]))
    use_b3 = bool(np.any(host["b3"]))
    use_g3 = not (np.allclose(host["g3"], 1.0) and not np.any(host["be3"]))
    use_b4 = bool(np.any(host["b4"]))
    use_g4 = not (np.allclose(host["g4"], 1.0) and not np.any(host["be4"]))

    nc = bass.Bass()
    t_fpx = nc.declare_dram_parameter("fpx", [NTOT, FPW], bf16, isOutput=False)
    t_ft = nc.declare_dram_parameter("ft", [D, NTOT], bf16, isOutput=False)
    t_w1b = nc.declare_dram_parameter("w1b", [D, CH], bf16, isOutput=False)
    t_wab = nc.declare_dram_parameter("wab", [128, H + 2], bf16, isOutput=False)
    t_wpk = nc.declare_dram_parameter("wpk", [128, PKW], f32, isOutput=False)
    t_wp2 = nc.declare_dram_parameter("wp2", [128, PK2W], f32, isOutput=False)
    t_out = nc.declare_dram_parameter("out", [GPC, D], f32, isOutput=True)
    t_om = nc.dram_tensor("om", [GPC * H, D + 1], f32)

    SRW = 33  # att raw rows 0..7, mean row 8, E[h^2] row 32

    with tile.TileContext(nc) as tc:
        with (
            tc.tile_pool(name="consts", bufs=1) as consts,
            tc.tile_pool(name="sb", bufs=6) as sb,
            tc.tile_pool(name="sbm", bufs=4) as sbm,
            tc.tile_pool(name="stats", bufs=4) as stats,
        ):
            # ---- constants ----
            wpk = consts.tile([128, PKW], f32)
            nc.sync.dma_start(out=wpk, in_=t_wpk[:, :])
            cwn = wpk[:, PK_CWN : PK_CWN + H]
            b1c = wpk[0:CH, PK_B1 : PK_B1 + 1]
            ident = wpk[:, PK_ID : PK_ID + 128]
            b2bc = wpk[:, PK_B2 : PK_B2 + H]
            w1b = consts.tile([D, CH], bf16)
            nc.sync.dma_start(out=w1b, in_=t_w1b[:, :])
            wab = consts.tile([128, H + 2], bf16)
            nc.sync.dma_start(out=wab, in_=t_wab[:, :])
            epsc = consts.tile([128, 1], f32)
            nc.vector.memset(epsc, EPS)
            zeroc = consts.tile([128, 1], f32)
            nc.vector.memset(zeroc, 0.0)

            with (
                tc.tile_pool(name="ph0", bufs=3, space=bass.MemorySpace.PSUM) as ph0,
                tc.tile_pool(name="pat", bufs=2, space=bass.MemorySpace.PSUM) as pat,
                tc.tile_pool(name="pm", bufs=3, space=bass.MemorySpace.PSUM) as pm,
                tc.tile_pool(name="wohp", bufs=8) as wohp,
                tc.tile_pool(name="hqp", bufs=6) as hqp,
            ):
                m_tiles = {}
                woh_tiles = {}
                fpx_tiles = {}
                hq_tiles = {}

                def bc(ap_base, step_g, n_inner, step_inner):
                    return bass.AP(
                        tensor=ap_base.tensor,
                        offset=ap_base.offset,
                        ap=[ap_base.ap[0], [step_g, G], [step_inner, n_inner]],
                    )

                def emit_loads(b):
                    ftt = sb.tile([D, G * 128], bf16, tag="ftt", name=f"ftt{b}")
                    nc.sync.dma_start(
                        out=ftt, in_=t_ft[:, b * G * 128 : (b + 1) * G * 128]
                    )
                    fpxt = sb.tile([128, G, FPW], bf16, tag="fpx", name=f"fpx{b}")
                    nc.sync.dma_start(
                        out=fpxt,
                        in_=t_fpx[b * G * 128 : (b + 1) * G * 128, :].rearrange(
                            "(i p) c -> p i c", p=128
                        ),
                    )
                    fpx_tiles[b] = (ftt, fpxt)

                def emit_h0(b):
                    # node MLP in 512-wide halves; hq stacks lrelu(h) on
                    # partitions 0:64 and its square on 64:128 so ONE matmul
                    # per chunk yields centered logits + mean + E[h^2]
                    ftt = fpx_tiles[b][0]
                    halves = []
                    for u in range(2):
                        h0h = ph0.tile([CH, 512], f32, tag="h0", name=f"h0_{b}_{u}")
                        nc.tensor.matmul(
                            h0h,
                            w1b,
                            ftt[:, u * 512 : (u + 1) * 512],
                            start=True,
                            stop=True,
                        )
                        hq = hqp.tile([128, 512], bf16, tag="hq", name=f"hq_{b}_{u}")
                        nc.scalar.activation(
                            hq[0:CH, :], h0h, AF.Prelu, bias=b1c, scale=1.0,
                            alpha=SLOPE,
                        )
                        nc.vector.tensor_mul(
                            hq[CH:128, :], hq[0:CH, :], hq[0:CH, :]
                        )
                        halves.append(hq)
                    hq_tiles[b] = halves

                def emit_att(b):
                    """at0 matmuls + stats + att + woh for batch b (consumes
                    hq produced one iteration earlier)."""
                    halves = hq_tiles.pop(b)
                    fpxt = fpx_tiles[b][1]
                    at0 = pat.tile([128, G, 12], f32, tag="at0", name=f"at0_{b}")
                    for g in range(G):
                        u, i = divmod(g, 4)
                        nc.tensor.matmul(
                            at0[:, g, 0:10],
                            halves[u][:, i * 128 : (i + 1) * 128],
                            wab[:, 0:10],
                            start=True,
                            stop=True,
                        )
                    # stats: rstd = exp(-0.5*ln(var+eps)) (no sqrt!)
                    stc = stats.tile([128, G], f32, tag="stc")
                    nc.vector.tensor_copy(stc, at0[:, :, H : H + 1])  # mu
                    st0 = stats.tile([128, G], f32, tag="st0")
                    nc.vector.tensor_mul(st0, stc, stc)  # mu^2
                    stv = stats.tile([128, G], f32, tag="stv")
                    nc.vector.tensor_sub(stv, at0[:, :, H + 1 : H + 2], st0)
                    stl = stats.tile([128, G], f32, tag="stl")
                    nc.scalar.activation(stl, stv, AF.Ln, bias=epsc, scale=1.0)
                    rstd = stats.tile([128, G], f32, tag="rstd")
                    nc.scalar.activation(rstd, stl, AF.Exp, bias=zeroc, scale=-0.5)
                    # wab cols 0..7 hold W2g - colsum(W2g)/CH, so at0 raw is
                    # already mean-centered: att2 = rstd * at0_raw.
                    att = stats.tile([128, G, H], bf16, tag="att")
                    att2 = stats.tile([128, G, H], f32, tag="att2")
                    nc.vector.tensor_mul(
                        att2, at0[:, :, 0:H], bc(rstd[:, 0:1], 1, H, 0)
                    )
                    if use_b2:
                        nc.vector.tensor_add(att2, att2, bc(b2bc[:, 0:1], 0, H, 1))
                    nc.scalar.activation(att, att2, AF.Exp, bias=zeroc)
                    for u in range(2):
                        wohh = wohp.tile(
                            [128, 4, WG, H], bf16, tag="woh", name=f"woh_{b}_{u}"
                        )
                        c_base = fpxt[:, u * 4 : u * 4 + 4, D + 1 : D + 1 + WG]
                        a_base = att[:, u * 4 : u * 4 + 4, :]
                        nc.vector.tensor_mul(
                            wohh,
                            bass.AP(
                                tensor=c_base.tensor,
                                offset=c_base.offset,
                                ap=[c_base.ap[0], [FPW, 4], [1, WG], [0, H]],
                            ),
                            bass.AP(
                                tensor=a_base.tensor,
                                offset=a_base.offset,
                                ap=[a_base.ap[0], [H, 4], [0, WG], [1, H]],
                            ),
                        )
                        woh_tiles[(b, u)] = wohh

                def emit_seg(bb):
                    """Segment matmuls + window finalize for batch bb."""
                    fpxt = fpx_tiles.pop(bb)[1]
                    for u in range(2):
                        wohh = woh_tiles.pop((bb, u))
                        for i in range(4):
                            c = bb * G + u * 4 + i
                            w = c // CPW
                            r = c % CPW
                            if r == 0:
                                m_tiles[w] = pm.tile(
                                    [128, D + 1], f32, tag="m", name=f"m{w}"
                                )
                            M = m_tiles[w]
                            nc.tensor.matmul(
                                M,
                                wohh[:, i, :, :],
                                fpxt[:, u * 4 + i, 0 : D + 1],
                                start=(r == 0),
                                stop=(r == CPW - 1),
                            )
                            if r == CPW - 1:
                                mo = sbm.tile([128, D + 1], f32, tag="mo")
                                nc.scalar.copy(mo, M)
                                nc.gpsimd.dma_start(
                                    out=t_om[w * 128 : (w + 1) * 128, :],
                                    in_=mo,
                                )
                                del m_tiles[w]

                # software pipeline: loads 1 ahead; at0/att 1 behind h0;
                # segment matmuls 2 behind att (3 behind h0).
                emit_loads(0)
                for b in range(NB):
                    if b + 1 < NB:
                        emit_loads(b + 1)
                    emit_h0(b)
                    if b >= 3:
                        emit_seg(b - 3)
                    if b >= 1:
                        emit_att(b - 1)
                emit_att(NB - 1)
                for bb in (NB - 3, NB - 2, NB - 1):
                    if bb >= 0:
                        emit_seg(bb)

            # ================= phase C: output MLP =================
            wp2 = consts.tile([128, PK2W], f32)
            nc.sync.dma_start(out=wp2, in_=t_wp2[:, :])
            w4sb = wp2[:, H * D : H * D + D]
            vec0 = H * D + D
            # vec rows: 0=b3, 1=g3, 2=beta3, 3=b4, 4=g4, 5=beta4 (broadcast)

            with (
                tc.tile_pool(name="pcT", bufs=2, space=bass.MemorySpace.PSUM) as pcT,
                tc.tile_pool(name="pcM", bufs=1, space=bass.MemorySpace.PSUM) as pcM,
            ):
                for t in range(GPC // 128):
                    omt = sb.tile([128, H, D + 1], f32, tag="omt")
                    nc.gpsimd.dma_start(
                        out=omt,
                        in_=t_om[t * 128 * H : (t + 1) * 128 * H, :].rearrange(
                            "(g h) c -> g h c", h=H
                        ),
                    )
                    smax = stats.tile([128, H], f32, tag="smax")
                    nc.vector.tensor_scalar_max(smax, omt[:, :, D : D + 1], 1e-30)
                    sinv = stats.tile([128, H], f32, tag="sinv")
                    nc.vector.reciprocal(sinv, smax)
                    omn = sbm.tile([128, H, D], f32, tag="omn")
                    nc.vector.tensor_mul(
                        omn,
                        bass.AP(
                            tensor=omt.tensor,
                            offset=omt.offset,
                            ap=[omt.ap[0], [D + 1, H], [1, D]],
                        ),
                        bass.AP(
                            tensor=sinv.tensor,
                            offset=sinv.offset,
                            ap=[sinv.ap[0], [1, H], [0, D]],
                        ),
                    )
                    omT = pcT.tile([128, H, D], f32, tag="omT")
                    for j in range(H):
                        nc.tensor.transpose(omT[:, j, :], omn[:, j, :], ident)
                    omTs = sbm.tile([128, H, D], f32, tag="omTs")
                    nc.scalar.copy(omTs, omT)
                    o2 = pcM.tile([128, D], f32, tag="o2")
                    for j in range(H):
                        nc.tensor.matmul(
                            o2,
                            omTs[:, j, :],
                            wp2[:, j * D : (j + 1) * D],
                            start=(j == 0),
                            stop=(j == H - 1),
                        )
                    o2s = sbm.tile([128, D], f32, tag="o2s")
                    nc.scalar.activation(o2s, o2, AF.Prelu, bias=zeroc, alpha=SLOPE)
                    if use_b3:
                        o2b = sbm.tile([128, D], f32, tag="o2b")
                        nc.vector.tensor_add(o2b, o2, wp2[:, vec0 : vec0 + D])
                        nc.vector.scalar_tensor_tensor(
                            o2s, o2b, SLOPE, o2b, OP.mult, OP.max
                        )
                    o2n = _ln_tile(nc, stats, sbm, o2s, "c3", epsc, zeroc)
                    if use_g3:
                        nc.vector.tensor_mul(o2n, o2n, wp2[:, vec0 + D : vec0 + 2 * D])
                        nc.vector.tensor_add(
                            o2n, o2n, wp2[:, vec0 + 2 * D : vec0 + 3 * D]
                        )
                    oT = pcM.tile([128, D], f32, tag="oT")
                    nc.tensor.transpose(oT, o2n, ident)
                    oTs = sbm.tile([128, D], f32, tag="oTs")
                    nc.scalar.copy(oTs, oT)
                    o3 = pcM.tile([128, D], f32, tag="o3")
                    nc.tensor.matmul(o3, oTs, w4sb, start=True, stop=True)
                    o3s = sbm.tile([128, D], f32, tag="o3s")
                    nc.scalar.activation(o3s, o3, AF.Prelu, bias=zeroc, alpha=SLOPE)
                    if use_b4:
                        o3b = sbm.tile([128, D], f32, tag="o3b")
                        nc.vector.tensor_add(
                            o3b, o3, wp2[:, vec0 + 3 * D : vec0 + 4 * D]
                        )
                        nc.vector.scalar_tensor_tensor(
                            o3s, o3b, SLOPE, o3b, OP.mult, OP.max
                        )
                    o3n = _ln_tile(nc, stats, sbm, o3s, "c4", epsc, zeroc)
                    if use_g4:
                        nc.vector.tensor_mul(
                            o3n, o3n, wp2[:, vec0 + 4 * D : vec0 + 5 * D]
                        )
                        nc.vector.tensor_add(
                            o3n, o3n, wp2[:, vec0 + 5 * D : vec0 + 6 * D]
                        )
                    nc.sync.dma_start(out=t_out[t * 128 : (t + 1) * 128, :], in_=o3n)
    return nc


def _ln_tile(nc, stats, sbm, x, tag, epsc, zeroc):
    """LayerNorm along free dim of x [128, D] -> new SBUF tile."""
    from concourse import mybir

    f32 = mybir.dt.float32
    AF = mybir.ActivationFunctionType
    OP = mybir.AluOpType
    bn = stats.tile([128, 6], f32, tag=tag + "bn")
    nc.vector.bn_stats(out=bn, in_=x)
    mv = stats.tile([128, 2], f32, tag=tag + "mv")
    nc.vector.bn_aggr(out=mv, in_=bn)
    sl = stats.tile([128, 1], f32, tag=tag + "sl")
    nc.scalar.activation(sl, mv[:, 1:2], AF.Ln, bias=epsc, scale=1.0)
    rs = stats.tile([128, 1], f32, tag=tag + "rs")
    nc.scalar.activation(rs, sl, AF.Exp, bias=zeroc, scale=-0.5)
    out = sbm.tile([128, x.shape[-1]], f32, tag=tag + "o")
    nc.vector.tensor_scalar(out, x, mv[:, 0:1], rs, OP.subtract, OP.mult)
    return out


def _split_multiwaits(nc):
    """Walrus here encodes at most one semaphore wait per instruction; move
    extra waits onto standalone InstEventSemaphore carriers inserted before
    the instruction (same engine stream, so ordering is preserved)."""
    from concourse import mybir

    ctr = 0
    for f in nc.m.functions:
        for blk in f.blocks:
            live = blk.instructions
            snapshot = list(live)
            live.clear()
            for inst in snapshot:
                si = inst.sync_info
                if si is not None and len(si.on_wait) > 1:
                    waits = list(si.on_wait)
                    for w in waits[:-1]:
                        ctr += 1
                        car = mybir.InstEventSemaphore(
                            name=f"WC-{ctr}", ins=[], outs=[]
                        )
                        car.engine = inst.engine
                        car.sync_info = mybir.SyncInfo(on_wait=[w], on_update=[])
                        live.append(car)
                    inst.sync_info = mybir.SyncInfo(
                        on_wait=[waits[-1]], on_update=list(si.on_update)
                    )
                live.append(inst)
    return ctr


def kernel(feat, batch, W1, b1, g1, beta1, W2, b2, W3, b3, g3, beta3, W4, b4,
           g4, beta4):
    global LAST_RESULT
    import ml_dtypes
    from concourse.bass_utils import run_bass_kernel_spmd

    bf16 = ml_dtypes.bfloat16
    feat = np.asarray(feat, np.float32)
    seg = np.asarray(batch).astype(np.int64)
    W1 = np.asarray(W1, np.float32)
    b1 = np.asarray(b1, np.float32)
    g1 = np.asarray(g1, np.float32)
    beta1 = np.asarray(beta1, np.float32)
    W2 = np.asarray(W2, np.float32)
    b2 = np.asarray(b2, np.float32)

    W2g = W2 * g1[:, None]
    b2p = b2 + beta1 @ W2  # [H]

    wab = np.zeros((128, H + 2), np.float32)
    wab[0:CH, 0:H] = W2g - W2g.sum(axis=0)[None, :] / CH  # fold mean-centering
    wab[0:CH, H] = 1.0 / CH       # mean from the hs rows
    wab[CH:128, H + 1] = 1.0 / CH  # E[h^2] from the hs^2 rows

    wpk = np.zeros((128, PKW), np.float32)
    wpk[:, PK_CWN : PK_CWN + H] = -W2g.sum(axis=0)[None, :]
    wpk[0:CH, PK_B1] = b1
    wpk[:, PK_ID : PK_ID + 128] = np.eye(128, dtype=np.float32)
    wpk[:, PK_B2 : PK_B2 + H] = b2p[None, :]

    W3m = np.asarray(W3, np.float32).reshape(H, 128, D)  # [j, k, e]
    wp2 = np.zeros((128, H * D + D + 6 * D), np.float32)
    wp2[:, 0 : H * D] = np.transpose(W3m, (1, 0, 2)).reshape(128, H * D)
    wp2[:, H * D : H * D + D] = np.asarray(W4, np.float32)
    vec0 = H * D + D
    for i, v in enumerate((b3, g3, beta3, b4, g4, beta4)):
        wp2[:, vec0 + i * D : vec0 + (i + 1) * D] = np.asarray(v, np.float32)[None, :]

    fpx_all, ft_all, NW, NTOT = _prep_shards(feat, seg, bf16)
    host = {
        "b2p": b2p,
        "b3": np.asarray(b3, np.float32),
        "g3": np.asarray(g3, np.float32),
        "be3": np.asarray(beta3, np.float32),
        "b4": np.asarray(b4, np.float32),
        "g4": np.asarray(g4, np.float32),
        "be4": np.asarray(beta4, np.float32),
    }
    nc = _build_program(NW, NTOT, host)
    _split_multiwaits(nc)

    common = {
        "w1b": W1.astype(bf16),
        "wab": wab.astype(bf16),
        "wpk": wpk,
        "wp2": wp2,
    }
    in_maps = [
        {"fpx": fpx_all[k], "ft": ft_all[k], **common} for k in range(NC_CORES)
    ]
    trace = bool(int(os.environ.get("BASS_KERNEL_TRACE", "0")))
    tmpdir = os.environ.get("BASS_KERNEL_TMPDIR") or None
    res = run_bass_kernel_spmd(
        nc, in_maps, list(range(NC_CORES)), trace=trace, tmpdir=tmpdir
    )
    LAST_RESULT = res
    out = np.concatenate(
        [np.asarray(res.results[k]["out"]) for k in range(NC_CORES)], axis=0
    )
    return out.astype(np.float32)


# revision 28
# speedup vs baseline: 1.0434x; 1.0434x over previous
"""Trainium2 Bass kernel for attention pooling over graph segments.

Reference computation (per node i with segment b = batch[i]):
    h   = LN(leaky_relu(feat @ W1 + b1)) * g1 + beta1
    att = exp(h @ W2 + b2)
    s_b = segment_sum(att);  att_n = att / s_b
    out_b = segment_sum(att_n[:, :, None] * feat[:, None, :])   # [B, H, D]
    o = LN(lrelu(out @ W3 + b3)) ; o = LN(lrelu(o @ W4 + b4))

Strategy (8 cores, data parallel by graph):
  - 512 graphs per core, grouped into 32 windows of 16 graphs.
  - Nodes are re-packed on host so each window's nodes start 128-aligned
    (padded with zero rows). A one-hot "window-local graph id" C[n, 0:16]
    plus a ones column are packed after the 128 feat columns.
  - att normalization folds into the output: out_b = (sum att*feat)/s_b.
  - Per 128-node chunk, lhsT = C (x) att (weighted one-hot [128, 16*8]) and
    one matmul against [feat | ones] accumulates both sum(att*feat) and s.

This walrus encodes at most one semaphore wait per instruction;
_split_multiwaits() hoists extra waits onto InstEventSemaphore carriers.

Perf notes: fp32 matmuls stream at 2 cycles/column, bf16 at 1 — the
node-MLP and segment matmuls run in bf16 (f32 PSUM accumulation).  All
ScalarE functions used (parametric relu, copy, ln, exp, square) live in
one activation-table set, and rstd = exp(-0.5*ln(var+eps)) keeps sqrt
off the engine, so the ~1.3us ACT_TABLE_LOAD never recurs.
"""

import os
import sys

import numpy as np

try:  # make concourse importable in bare environments
    import concourse  # noqa: F401
except ImportError:  # pragma: no cover
    sys.path.insert(0, "/opt/trn_rl_repo")

NUM_GRAPHS = 4096
NC_CORES = 8
WG = 16  # graphs per window
D = 128
H = 8
CH = 64  # hidden channels
EPS = 1e-6
SLOPE = 0.01

# packed f32 constant column offsets (wpk, [128, PKW])
PK_CWN = 0         # [128, 8]  -colsum(W2g), broadcast
PK_B1 = 8          # [64, 1]
PK_ID = 9          # [128, 128] identity
PK_B2 = 137        # [128, 8]  b2 + beta1@W2, broadcast
PKW = 145

LAST_RESULT = None  # BassKernelResults of the most recent run (for test.py)


def _prep_shards(feat, seg, bf16):
    """Window-pad nodes per core; build fpx = [feat | ones | C] and feat^T."""
    bounds = np.searchsorted(seg, np.arange(NUM_GRAPHS + 1))
    wstart = bounds[::WG]  # 257 entries
    wcnt = np.diff(wstart)
    nwin_per_core = NUM_GRAPHS // WG // NC_CORES  # 32
    NW = max(128, int(-(-int(wcnt.max()) // 128)) * 128)
    NTOT = nwin_per_core * NW
    fpx_all = []
    ft_all = []
    for k in range(NC_CORES):
        fpx = np.zeros((NTOT, D + 1 + WG), np.float32)
        fpx[:, D] = 1.0  # ones column (harmless on pad rows; C gates them)
        for j in range(nwin_per_core):
            w = k * nwin_per_core + j
            s, e = int(wstart[w]), int(wstart[w + 1])
            n = e - s
            if n == 0:
                continue
            fpx[j * NW : j * NW + n, :D] = feat[s:e]
            gl = (seg[s:e] - w * WG).astype(np.int64)
            fpx[j * NW + np.arange(n), D + 1 + gl] = 1.0
        ft = np.ascontiguousarray(fpx[:, :D].T).astype(bf16)
        fpx_all.append(fpx.astype(bf16))
        ft_all.append(ft)
    return fpx_all, ft_all, NW, NTOT


def _build_program(NW, NTOT, host):
    import concourse.bass as bass
    import concourse.tile as tile
    from concourse import mybir

    f32 = mybir.dt.float32
    bf16 = mybir.dt.bfloat16
    AF = mybir.ActivationFunctionType
    OP = mybir.AluOpType

    GPC = NUM_GRAPHS // NC_CORES  # 512 graphs per core
    CPW = NW // 128  # chunks per window
    NCHUNK = NTOT // 128
    G = 8  # chunks per batch (1024 nodes)
    NB = NCHUNK // G
    FPW = D + 1 + WG  # fpx row width: feat | ones | C
    PK2W = H * D + D + 6 * D  # w3 | w4 | 6 broadcast vectors

    use_b2 = bool(np.any(host["b2p"]))
    use_b3 = bool(np.any(host["b3"]))
    use_g3 = not (np.allclose(host["g3"], 1.0) and not np.any(host["be3"]))
    use_b4 = bool(np.any(host["b4"]))
    use_g4 = not (np.allclose(host["g4"], 1.0) and not np.any(host["be4"]))

    nc = bass.Bass()
    t_fpx = nc.declare_dram_parameter("fpx", [NTOT, FPW], bf16, isOutput=False)
    t_ft = nc.declare_dram_parameter("ft", [D, NTOT], bf16, isOutput=False)
    t_w1b = nc.declare_dram_parameter("w1b", [D, CH], bf16, isOutput=False)
    t_wab = nc.declare_dram_parameter("wab", [128, H + 2], bf16, isOutput=False)
    t_wpk = nc.declare_dram_parameter("wpk", [128, PKW], f32, isOutput=False)
    t_wp2 = nc.declare_dram_parameter("wp2", [128, PK2W], f32, isOutput=False)
    t_out = nc.declare_dram_parameter("out", [GPC, D], f32, isOutput=True)
    t_om = nc.dram_tensor("om", [GPC, H * D], f32)

    SRW = 33  # att raw rows 0..7, mean row 8, E[h^2] row 32

    with tile.TileContext(nc) as tc:
        with (
            tc.tile_pool(name="consts", bufs=1) as consts,
            tc.tile_pool(name="sb", bufs=6) as sb,
            tc.tile_pool(name="sbm", bufs=4) as sbm,
            tc.tile_pool(name="stats", bufs=4) as stats,
        ):
            # ---- constants ----
            wpk = consts.tile([128, PKW], f32)
            nc.sync.dma_start(out=wpk, in_=t_wpk[:, :])
            cwn = wpk[:, PK_CWN : PK_CWN + H]
            b1c = wpk[0:CH, PK_B1 : PK_B1 + 1]
            ident = wpk[:, PK_ID : PK_ID + 128]
            b2bc = wpk[:, PK_B2 : PK_B2 + H]
            w1b = consts.tile([D, CH], bf16)
            nc.sync.dma_start(out=w1b, in_=t_w1b[:, :])
            wab = consts.tile([128, H + 2], bf16)
            nc.sync.dma_start(out=wab, in_=t_wab[:, :])
            epsc = consts.tile([128, 1], f32)
            nc.vector.memset(epsc, EPS)
            zeroc = consts.tile([128, 1], f32)
            nc.vector.memset(zeroc, 0.0)

            with (
                tc.tile_pool(name="ph0", bufs=3, space=bass.MemorySpace.PSUM) as ph0,
                tc.tile_pool(name="pat", bufs=2, space=bass.MemorySpace.PSUM) as pat,
                tc.tile_pool(name="pm", bufs=3, space=bass.MemorySpace.PSUM) as pm,
                tc.tile_pool(name="wohp", bufs=8) as wohp,
                tc.tile_pool(name="hqp", bufs=6) as hqp,
            ):
                m_tiles = {}
                woh_tiles = {}
                fpx_tiles = {}
                hq_tiles = {}

                def bc(ap_base, step_g, n_inner, step_inner):
                    return bass.AP(
                        tensor=ap_base.tensor,
                        offset=ap_base.offset,
                        ap=[ap_base.ap[0], [step_g, G], [step_inner, n_inner]],
                    )

                def emit_loads(b):
                    ftt = sb.tile([D, G * 128], bf16, tag="ftt", name=f"ftt{b}")
                    nc.sync.dma_start(
                        out=ftt, in_=t_ft[:, b * G * 128 : (b + 1) * G * 128]
                    )
                    fpxt = sb.tile([128, G, FPW], bf16, tag="fpx", name=f"fpx{b}")
                    nc.sync.dma_start(
                        out=fpxt,
                        in_=t_fpx[b * G * 128 : (b + 1) * G * 128, :].rearrange(
                            "(i p) c -> p i c", p=128
                        ),
                    )
                    fpx_tiles[b] = (ftt, fpxt)

                def emit_h0(b):
                    # node MLP in 512-wide halves; hq stacks lrelu(h) on
                    # partitions 0:64 and its square on 64:128 so ONE matmul
                    # per chunk yields centered logits + mean + E[h^2]
                    ftt = fpx_tiles[b][0]
                    halves = []
                    for u in range(2):
                        h0h = ph0.tile([CH, 512], f32, tag="h0", name=f"h0_{b}_{u}")
                        nc.tensor.matmul(
                            h0h,
                            w1b,
                            ftt[:, u * 512 : (u + 1) * 512],
                            start=True,
                            stop=True,
                        )
                        hq = hqp.tile([128, 512], bf16, tag="hq", name=f"hq_{b}_{u}")
                        nc.scalar.activation(
                            hq[0:CH, :], h0h, AF.Prelu, bias=b1c, scale=1.0,
                            alpha=SLOPE,
                        )
                        nc.vector.tensor_mul(
                            hq[CH:128, :], hq[0:CH, :], hq[0:CH, :]
                        )
                        halves.append(hq)
                    hq_tiles[b] = halves

                def emit_att(b):
                    """at0 matmuls + stats + att + woh for batch b (consumes
                    hq produced one iteration earlier)."""
                    halves = hq_tiles.pop(b)
                    fpxt = fpx_tiles[b][1]
                    at0 = pat.tile([128, G, 12], f32, tag="at0", name=f"at0_{b}")
                    for g in range(G):
                        u, i = divmod(g, 4)
                        nc.tensor.matmul(
                            at0[:, g, 0:10],
                            halves[u][:, i * 128 : (i + 1) * 128],
                            wab[:, 0:10],
                            start=True,
                            stop=True,
                        )
                    # stats: rstd = exp(-0.5*ln(var+eps)) (no sqrt!)
                    stc = stats.tile([128, G], f32, tag="stc")
                    nc.vector.tensor_copy(stc, at0[:, :, H : H + 1])  # mu
                    st0 = stats.tile([128, G], f32, tag="st0")
                    nc.vector.tensor_mul(st0, stc, stc)  # mu^2
                    stv = stats.tile([128, G], f32, tag="stv")
                    nc.vector.tensor_sub(stv, at0[:, :, H + 1 : H + 2], st0)
                    stl = stats.tile([128, G], f32, tag="stl")
                    nc.scalar.activation(stl, stv, AF.Ln, bias=epsc, scale=1.0)
                    rstd = stats.tile([128, G], f32, tag="rstd")
                    nc.scalar.activation(rstd, stl, AF.Exp, bias=zeroc, scale=-0.5)
                    # wab cols 0..7 hold W2g - colsum(W2g)/CH, so at0 raw is
                    # already mean-centered: att2 = rstd * at0_raw.
                    att = stats.tile([128, G, H], bf16, tag="att")
                    att2 = stats.tile([128, G, H], f32, tag="att2")
                    nc.vector.tensor_mul(
                        att2, at0[:, :, 0:H], bc(rstd[:, 0:1], 1, H, 0)
                    )
                    if use_b2:
                        nc.vector.tensor_add(att2, att2, bc(b2bc[:, 0:1], 0, H, 1))
                    nc.scalar.activation(att, att2, AF.Exp, bias=zeroc)
                    for u in range(2):
                        wohh = wohp.tile(
                            [128, 4, WG, H], bf16, tag="woh", name=f"woh_{b}_{u}"
                        )
                        c_base = fpxt[:, u * 4 : u * 4 + 4, D + 1 : D + 1 + WG]
                        a_base = att[:, u * 4 : u * 4 + 4, :]
                        nc.vector.tensor_mul(
                            wohh,
                            bass.AP(
                                tensor=c_base.tensor,
                                offset=c_base.offset,
                                ap=[c_base.ap[0], [FPW, 4], [1, WG], [0, H]],
                            ),
                            bass.AP(
                                tensor=a_base.tensor,
                                offset=a_base.offset,
                                ap=[a_base.ap[0], [H, 4], [0, WG], [1, H]],
                            ),
                        )
                        woh_tiles[(b, u)] = wohh

                def emit_seg(bb):
                    """Segment matmuls + window finalize for batch bb."""
                    fpxt = fpx_tiles.pop(bb)[1]
                    for u in range(2):
                        wohh = woh_tiles.pop((bb, u))
                        for i in range(4):
                            c = bb * G + u * 4 + i
                            w = c // CPW
                            r = c % CPW
                            if r == 0:
                                m_tiles[w] = pm.tile(
                                    [128, D + 1], f32, tag="m", name=f"m{w}"
                                )
                            M = m_tiles[w]
                            nc.tensor.matmul(
                                M,
                                wohh[:, i, :, :],
                                fpxt[:, u * 4 + i, 0 : D + 1],
                                start=(r == 0),
                                stop=(r == CPW - 1),
                            )
                            if r == CPW - 1:
                                sm = stats.tile([128, 1], f32, tag="sm")
                                nc.vector.tensor_scalar_max(
                                    sm, M[:, D : D + 1], 1e-30
                                )
                                sr_ = stats.tile([128, 1], f32, tag="sr_")
                                nc.vector.reciprocal(sr_, sm)
                                mo = sbm.tile([128, D], f32, tag="mo")
                                nc.vector.tensor_scalar_mul(mo, M[:, 0:D], sr_)
                                nc.gpsimd.dma_start(
                                    out=t_om[w * WG : (w + 1) * WG, :].rearrange(
                                        "g (h d) -> (g h) d", h=H
                                    ),
                                    in_=mo,
                                )
                                del m_tiles[w]

                # software pipeline: loads 1 ahead; segment matmuls lag 2
                # batches so PE has ready work while the activation chain of
                # the current batch is in flight.
                emit_loads(0)
                for b in range(NB):
                    if b + 1 < NB:
                        emit_loads(b + 1)
                    emit_h0(b)
                    if b >= 2:
                        emit_seg(b - 2)
                    emit_att(b)
                for bb in (NB - 2, NB - 1):
                    if bb >= 0:
                        emit_seg(bb)

            # ================= phase C: output MLP =================
            wp2 = consts.tile([128, PK2W], f32)
            nc.sync.dma_start(out=wp2, in_=t_wp2[:, :])
            w4sb = wp2[:, H * D : H * D + D]
            vec0 = H * D + D
            # vec rows: 0=b3, 1=g3, 2=beta3, 3=b4, 4=g4, 5=beta4 (broadcast)

            with (
                tc.tile_pool(name="pcT", bufs=2, space=bass.MemorySpace.PSUM) as pcT,
                tc.tile_pool(name="pcM", bufs=1, space=bass.MemorySpace.PSUM) as pcM,
            ):
                for t in range(GPC // 128):
                    omt = sb.tile([128, H * D], f32, tag="omt")
                    nc.gpsimd.dma_start(
                        out=omt, in_=t_om[t * 128 : (t + 1) * 128, :]
                    )
                    omT = pcT.tile([128, H, D], f32, tag="omT")
                    for j in range(H):
                        nc.tensor.transpose(
                            omT[:, j, :], omt[:, j * 128 : (j + 1) * 128], ident
                        )
                    omTs = sbm.tile([128, H, D], f32, tag="omTs")
                    nc.scalar.copy(omTs, omT)
                    o2 = pcM.tile([128, D], f32, tag="o2")
                    for j in range(H):
                        nc.tensor.matmul(
                            o2,
                            omTs[:, j, :],
                            wp2[:, j * D : (j + 1) * D],
                            start=(j == 0),
                            stop=(j == H - 1),
                        )
                    o2s = sbm.tile([128, D], f32, tag="o2s")
                    nc.scalar.activation(o2s, o2, AF.Prelu, bias=zeroc, alpha=SLOPE)
                    if use_b3:
                        o2b = sbm.tile([128, D], f32, tag="o2b")
                        nc.vector.tensor_add(o2b, o2, wp2[:, vec0 : vec0 + D])
                        nc.vector.scalar_tensor_tensor(
                            o2s, o2b, SLOPE, o2b, OP.mult, OP.max
                        )
                    o2n = _ln_tile(nc, stats, sbm, o2s, "c3", epsc, zeroc)
                    if use_g3:
                        nc.vector.tensor_mul(o2n, o2n, wp2[:, vec0 + D : vec0 + 2 * D])
                        nc.vector.tensor_add(
                            o2n, o2n, wp2[:, vec0 + 2 * D : vec0 + 3 * D]
                        )
                    oT = pcM.tile([128, D], f32, tag="oT")
                    nc.tensor.transpose(oT, o2n, ident)
                    oTs = sbm.tile([128, D], f32, tag="oTs")
                    nc.scalar.copy(oTs, oT)
                    o3 = pcM.tile([128, D], f32, tag="o3")
                    nc.tensor.matmul(o3, oTs, w4sb, start=True, stop=True)
                    o3s = sbm.tile([128, D], f32, tag="o3s")
                    nc.scalar.activation(o3s, o3, AF.Prelu, bias=zeroc, alpha=SLOPE)
                    if use_b4:
                        o3b = sbm.tile([128, D], f32, tag="o3b")
                        nc.vector.tensor_add(
                            o3b, o3, wp2[:, vec0 + 3 * D : vec0 + 4 * D]
                        )
                        nc.vector.scalar_tensor_tensor(
                            o3s, o3b, SLOPE, o3b, OP.mult, OP.max
                        )
                    o3n = _ln_tile(nc, stats, sbm, o3s, "c4", epsc, zeroc)
                    if use_g4:
                        nc.vector.tensor_mul(
                            o3n, o3n, wp2[:, vec0 + 4 * D : vec0 + 5 * D]
                        )
                        nc.vector.tensor_add(
                            o3n, o3n, wp2[:, vec0 + 5 * D : vec0 + 6 * D]
                        )
                    nc.sync.dma_start(out=t_out[t * 128 : (t + 1) * 128, :], in_=o3n)
    return nc


def _ln_tile(nc, stats, sbm, x, tag, epsc, zeroc):
    """LayerNorm along free dim of x [128, D] -> new SBUF tile."""
    from concourse import mybir

    f32 = mybir.dt.float32
    AF = mybir.ActivationFunctionType
    OP = mybir.AluOpType
    bn = stats.tile([128, 6], f32, tag=tag + "bn")
    nc.vector.bn_stats(out=bn, in_=x)
    mv = stats.tile([128, 2], f32, tag=tag + "mv")
    nc.vector.bn_aggr(out=mv, in_=bn)
    sl = stats.tile([128, 1], f32, tag=tag + "sl")
    nc.scalar.activation(sl, mv[:, 1:2], AF.Ln, bias=epsc, scale=1.0)
    rs = stats.tile([128, 1], f32, tag=tag + "rs")
    nc.scalar.activation(rs, sl, AF.Exp, bias=zeroc, scale=-0.5)
    out = sbm.tile([128, x.shape[-1]], f32, tag=tag + "o")
    nc.vector.tensor_scalar(out, x, mv[:, 0:1], rs, OP.subtract, OP.mult)
    return out


def _split_multiwaits(nc):
    """Walrus here encodes at most one semaphore wait per instruction; move
    extra waits onto standalone InstEventSemaphore carriers inserted before
    the instruction (same engine stream, so ordering is preserved)."""
    from concourse import mybir

    ctr = 0
    for f in nc.m.functions:
        for blk in f.blocks:
            live = blk.instructions
            snapshot = list(live)
            live.clear()
            for inst in snapshot:
                si = inst.sync_info
                if si is not None and len(si.on_wait) > 1:
                    waits = list(si.on_wait)
                    for w in waits[:-1]:
                        ctr += 1
                        car = mybir.InstEventSemaphore(
                            name=f"WC-{ctr}", ins=[], outs=[]
                        )
                        car.engine = inst.engine
                        car.sync_info = mybir.SyncInfo(on_wait=[w], on_update=[])
                        live.append(car)
                    inst.sync_info = mybir.SyncInfo(
                        on_wait=[waits[-1]], on_update=list(si.on_update)
                    )
                live.append(inst)
    return ctr


def kernel(feat, batch, W1, b1, g1, beta1, W2, b2, W3, b3, g3, beta3, W4, b4,
           g4, beta4):
    global LAST_RESULT
    import ml_dtypes
    from concourse.bass_utils import run_bass_kernel_spmd

    bf16 = ml_dtypes.bfloat16
    feat = np.asarray(feat, np.float32)
    seg = np.asarray(batch).astype(np.int64)
    W1 = np.asarray(W1, np.float32)
    b1 = np.asarray(b1, np.float32)
    g1 = np.asarray(g1, np.float32)
    beta1 = np.asarray(beta1, np.float32)
    W2 = np.asarray(W2, np.float32)
    b2 = np.asarray(b2, np.float32)

    W2g = W2 * g1[:, None]
    b2p = b2 + beta1 @ W2  # [H]

    wab = np.zeros((128, H + 2), np.float32)
    wab[0:CH, 0:H] = W2g - W2g.sum(axis=0)[None, :] / CH  # fold mean-centering
    wab[0:CH, H] = 1.0 / CH       # mean from the hs rows
    wab[CH:128, H + 1] = 1.0 / CH  # E[h^2] from the hs^2 rows

    wpk = np.zeros((128, PKW), np.float32)
    wpk[:, PK_CWN : PK_CWN + H] = -W2g.sum(axis=0)[None, :]
    wpk[0:CH, PK_B1] = b1
    wpk[:, PK_ID : PK_ID + 128] = np.eye(128, dtype=np.float32)
    wpk[:, PK_B2 : PK_B2 + H] = b2p[None, :]

    W3m = np.asarray(W3, np.float32).reshape(H, 128, D)  # [j, k, e]
    wp2 = np.zeros((128, H * D + D + 6 * D), np.float32)
    wp2[:, 0 : H * D] = np.transpose(W3m, (1, 0, 2)).reshape(128, H * D)
    wp2[:, H * D : H * D + D] = np.asarray(W4, np.float32)
    vec0 = H * D + D
    for i, v in enumerate((b3, g3, beta3, b4, g4, beta4)):
        wp2[:, vec0 + i * D : vec0 + (i + 1) * D] = np.asarray(v, np.float32)[None, :]

    fpx_all, ft_all, NW, NTOT = _prep_shards(feat, seg, bf16)
    host = {
        "b2p": b2p,
        "b3": np.asarray(b3, np.float32),
        "g3": np.asarray(g3, np.float32),
        "be3": np.asarray(beta3, np.float32),
        "b4": np.asarray(b4, np.float32),
        "g4": np.asarray(g4, np.float32),
        "be4": np.asarray(beta4, np.float32),
    }
    nc = _build_program(NW, NTOT, host)
    _split_multiwaits(nc)

    common = {
        "w1b": W1.astype(bf16),
        "wab": wab.astype(bf16),
        "wpk": wpk,
        "wp2": wp2,
    }
    in_maps = [
        {"fpx": fpx_all[k], "ft": ft_all[k], **common} for k in range(NC_CORES)
    ]
    trace = bool(int(os.environ.get("BASS_KERNEL_TRACE", "0")))
    tmpdir = os.environ.get("BASS_KERNEL_TMPDIR") or None
    res = run_bass_kernel_spmd(
        nc, in_maps, list(range(NC_CORES)), trace=trace, tmpdir=tmpdir
    )
    LAST_RESULT = res
    out = np.concatenate(
        [np.asarray(res.results[k]["out"]) for k in range(NC_CORES)], axis=0
    )
    return out.astype(np.float32)
